# revision 11
# baseline (speedup 1.0000x reference)
"""Causal self-attention (B=2, T=2048, C=1024, H=16) on 8 TRN2 NeuronCores.

Sharding: data-parallel on batch (2) x tensor-parallel on heads (4 groups of
4 heads) = 8 cores. Each core computes, for its batch b and head group g:
  QKV^T projection for its 256 qkv columns, causal flash-style attention for
  its 4 heads, and a partial output projection  Y_g @ W_proj[256g:256(g+1)].
The host sums the 4 partial projections per batch and adds b_proj.

On-chip dataflow (all matmuls in float32r ~ tf32), fully pipelined per
512-query block so PE-bound projection work overlaps ACT-bound softmax:
  xT   = transpose(x_b)                [C-part, tq]     (PE transpose)
  Q^T  = Wq.T @ x via lhsT=Wq chunks   [qcol-part, tq]
  K^T  likewise; V natural             [tk-part, vcol]
  S^T  = K^T_blk.T @ Q^T               [tk-part, tq]  (2 heads row-packed)
  E    = exp(S^T/8) (ACT, PSUM->SBUF), triangle mask on diagonal strips
  Ynum^T, denom = [V_h | 1].T @ E      [65-part, tq]  (PSUM accumulated)
  Y^T  = Ynum^T * (1/denom)            (approx recip + GPSIMD bcast + DVE)
  out += Y^T.T @ Wp                    [tq-part, cout]
"""

import numpy as np

import concourse.bacc as bacc
import concourse.mybir as mybir
from concourse import bass_utils
from concourse.bass import ts
from concourse.masks import make_identity
from concourse.tile import TileContext

P = 128
T = 2048
C = 1024
KO = C // P          # 8 contraction chunks over C
HC = 256             # qkv columns per core (4 heads x 64)
NH = 4               # heads per core
D = 64
NTK = T // P         # 16 key blocks
TQB = 512            # query block (free dim)
NQ = T // TQB        # 4 query blocks
SCALE = 1.0 / np.sqrt(D)

f32 = mybir.dt.float32
f32r = mybir.dt.float32r
AF = mybir.ActivationFunctionType
ALU = mybir.AluOpType

_NC = None


def _build():
    nc = bacc.Bacc(trn_type="TRN2", target_bir_lowering=False, debug=False)

    x_d = nc.dram_tensor("x", [T, C], f32, kind="ExternalInput")
    wq_d = nc.dram_tensor("wq", [C, HC], f32r, kind="ExternalInput")
    wk_d = nc.dram_tensor("wk", [C, HC], f32r, kind="ExternalInput")
    wv_d = nc.dram_tensor("wv", [C, HC], f32r, kind="ExternalInput")
    wp_d = nc.dram_tensor("wp", [HC, C], f32r, kind="ExternalInput")
    bq_d = nc.dram_tensor("bq", [HC], f32, kind="ExternalInput")
    bk_d = nc.dram_tensor("bk", [HC], f32, kind="ExternalInput")
    bv_d = nc.dram_tensor("bv", [HC], f32, kind="ExternalInput")
    out_d = nc.dram_tensor("out", [T, C], f32, kind="ExternalOutput")

    with TileContext(nc) as tc:
        with (
            tc.tile_pool(name="persist", bufs=1) as pp,
            tc.tile_pool(name="psum", bufs=2, space="PSUM") as ps,
            tc.tile_pool(name="xs", bufs=2) as xsp,
            tc.tile_pool(name="xn", bufs=3) as xnp,
            tc.tile_pool(name="e", bufs=6) as ep,
            tc.tile_pool(name="r", bufs=4) as rp,
            tc.tile_pool(name="o", bufs=3) as op,
        ):
            wq = pp.tile([P, KO, HC], f32r, tag="wq")
            wk = pp.tile([P, KO, HC], f32r, tag="wk")
            wv = pp.tile([P, KO, HC], f32r, tag="wv")
            wp = pp.tile([P, 2, C], f32r, tag="wp")
            bqt = pp.tile([P, 2], f32, tag="bqt")
            bkt = pp.tile([P, 2], f32, tag="bkt")
            bvt = pp.tile([P, HC], f32, tag="bvt")
            qt = pp.tile([P, 2, T], f32r, tag="qt")
            kt = pp.tile([P, 2, T], f32r, tag="kt")
            v = pp.tile([P, NTK, NH, D + 1], f32r, tag="v")
            yt = pp.tile([P, 2, T], f32r, tag="yt")
            mask = pp.tile([P, P], f32, tag="mask")
            ident = pp.tile([P, P], f32, tag="ident")

            nc.sync.dma_start(wq[:], wq_d.ap().rearrange("(ko p) n -> p ko n", p=P))
            nc.sync.dma_start(wk[:], wk_d.ap().rearrange("(ko p) n -> p ko n", p=P))
            nc.sync.dma_start(wv[:], wv_d.ap().rearrange("(ko p) n -> p ko n", p=P))
            nc.sync.dma_start(wp[:], wp_d.ap().rearrange("(kc p) n -> p kc n", p=P))
            nc.sync.dma_start(bqt[:], bq_d.ap().rearrange("(c p) -> p c", p=P))
            nc.sync.dma_start(bkt[:], bk_d.ap().rearrange("(c p) -> p c", p=P))
            nc.sync.dma_start(bvt[:], bv_d.ap()[None, :].to_broadcast((P, HC)))

            make_identity(nc, ident[:])
            # ones columns for the denominator rows of V_aug
            nc.gpsimd.memset(v[:].bitcast(f32), 1.0)
            # triangle mask: mask[p, f] = 1 iff p <= f
            nc.gpsimd.memset(mask[:], 1.0)
            nc.gpsimd.affine_select(
                out=mask[:],
                in_=mask[:],
                compare_op=ALU.is_ge,
                fill=0.0,
                base=0,
                pattern=[[1, P]],
                channel_multiplier=-1,
            )
            tri = mask[:, 0:P].bitcast(f32r)

            for tqb in range(NQ):
                # -- transpose this 512-row slice of x into xs[C-part, 512] --
                xs = xsp.tile([P, KO, TQB], f32r, tag="xs")
                for lt in range(4):
                    ti = 4 * tqb + lt
                    xn = xnp.tile([P, C], f32, tag="xn")
                    nc.sync.dma_start(xn[:], x_d.ap()[ts(ti, P), :])
                    for kk in range(0, KO, 4):
                        pt = ps.tile([P, 512], f32, tag="mm512")
                        for j in range(4):
                            nc.tensor.transpose(
                                pt[:, ts(j, P)], xn[:, ts(kk + j, P)], ident[:]
                            )
                        nc.vector.tensor_copy(
                            xs[:, kk : kk + 4, ts(lt, P)],
                            pt[:].rearrange("p (k t) -> p k t", k=4),
                        )

                # -- Q^T / K^T for this query block --
                for cc in range(2):
                    pq = ps.tile([P, TQB], f32, tag="mm512")
                    for ko in range(KO):
                        nc.tensor.matmul(
                            pq[:],
                            wq[:, ko, ts(cc, P)],
                            xs[:, ko, :],
                            start=(ko == 0),
                            stop=(ko == KO - 1),
                        )
                    nc.vector.tensor_scalar_add(
                        qt[:, cc, ts(tqb, TQB)], pq[:], bqt[:, cc : cc + 1]
                    )
                    pk = ps.tile([P, TQB], f32, tag="mm512")
                    for ko in range(KO):
                        nc.tensor.matmul(
                            pk[:],
                            wk[:, ko, ts(cc, P)],
                            xs[:, ko, :],
                            start=(ko == 0),
                            stop=(ko == KO - 1),
                        )
                    nc.vector.tensor_scalar_add(
                        kt[:, cc, ts(tqb, TQB)], pk[:], bkt[:, cc : cc + 1]
                    )

                # -- V for the 4 key blocks of this slice --
                for lt in range(4):
                    tk = 4 * tqb + lt
                    pv = ps.tile([P, HC], f32, tag="mm512")
                    for ko in range(KO):
                        nc.tensor.matmul(
                            pv[:],
                            xs[:, ko, ts(lt, P)],
                            wv[:, ko, :],
                            start=(ko == 0),
                            stop=(ko == KO - 1),
                        )
                    nc.vector.tensor_tensor(
                        v[:, tk, :, 0:D],
                        pv[:].rearrange("p (h a) -> p h a", h=NH),
                        bvt[:].rearrange("p (h a) -> p h a", h=NH),
                        ALU.add,
                    )

                # -- causal attention for both head pairs --
                for hp in range(2):
                    h0, h1 = 2 * hp, 2 * hp + 1
                    y0 = ps.tile([D + 1, TQB], f32, tag="y")
                    y1 = ps.tile([D + 1, TQB], f32, tag="y")
                    q0 = qt[0:64, hp, ts(tqb, TQB)]
                    q1 = qt[64:128, hp, ts(tqb, TQB)]
                    ntk = 4 * (tqb + 1)
                    for tkk in range(0, ntk, 2):
                        s0 = ps.tile([P, 2 * TQB], f32, tag="s")
                        s1 = ps.tile([P, 2 * TQB], f32, tag="s")
                        for j in range(2):
                            tk = tkk + j
                            nc.tensor.matmul(
                                s0[:, ts(j, TQB)],
                                kt[0:64, hp, ts(tk, P)],
                                q0,
                                start=True,
                                stop=True,
                                tile_position=(0, 0),
                            )
                            nc.tensor.matmul(
                                s1[:, ts(j, TQB)],
                                kt[64:128, hp, ts(tk, P)],
                                q1,
                                start=True,
                                stop=True,
                                tile_position=(64, 0),
                            )
                        e0 = ep.tile([P, 2 * TQB], f32r, tag="e")
                        e1 = ep.tile([P, 2 * TQB], f32r, tag="e")
                        nc.scalar.activation(e0[:], s0[:], AF.Exp, scale=SCALE)
                        nc.scalar.activation(e1[:], s1[:], AF.Exp, scale=SCALE)
                        for j in range(2):
                            tk = tkk + j
                            jd = tk - 4 * tqb  # diagonal strip index
                            if jd >= 0:
                                # cols < 128*jd are fully masked; the
                                # [128*jd, 128*(jd+1)) strip is triangular
                                for e in (e0, e1):
                                    if jd > 0:
                                        nc.vector.memset(
                                            e[:, j * TQB : j * TQB + jd * P].bitcast(
                                                f32
                                            ),
                                            0.0,
                                        )
                                    st = j * TQB + jd * P
                                    nc.vector.tensor_mul(
                                        e[:, st : st + P], e[:, st : st + P], tri
                                    )
                        for j in range(2):
                            tk = tkk + j
                            nc.tensor.matmul(
                                y0[:],
                                v[:, tk, h0, :],
                                e0[:, ts(j, TQB)],
                                start=(tk == 0),
                                stop=(tk == ntk - 1),
                            )
                            nc.tensor.matmul(
                                y1[:],
                                v[:, tk, h1, :],
                                e1[:, ts(j, TQB)],
                                start=(tk == 0),
                                stop=(tk == ntk - 1),
                            )
                    den0 = rp.tile([1, TQB], f32, tag="den")
                    den1 = rp.tile([1, TQB], f32, tag="den")
                    nc.vector.tensor_copy(den0[:], y0[64:65, :])
                    nc.vector.tensor_copy(den1[:], y1[64:65, :])
                    rec0 = rp.tile([1, TQB], f32, tag="rec")
                    rec1 = rp.tile([1, TQB], f32, tag="rec")
                    nc.vector.reciprocal_approx_fast(rec0[:], den0[:])
                    nc.vector.reciprocal_approx_fast(rec1[:], den1[:])
                    rb0 = rp.tile([D, TQB], f32, tag="rb")
                    rb1 = rp.tile([D, TQB], f32, tag="rb")
                    nc.gpsimd.partition_broadcast(rb0[:], rec0[:])
                    nc.gpsimd.partition_broadcast(rb1[:], rec1[:])
                    nc.vector.tensor_mul(
                        yt[0:64, hp, ts(tqb, TQB)], y0[0:64, :], rb0[:]
                    )
                    nc.vector.tensor_mul(
                        yt[64:128, hp, ts(tqb, TQB)], y1[0:64, :], rb1[:]
                    )

                # -- partial output projection for this query block --
                for mt in range(4 * tqb, 4 * tqb + 4):
                    for nb in range(2):
                        pj = ps.tile([P, 512], f32, tag="mm512")
                        for kc in range(2):
                            nc.tensor.matmul(
                                pj[:],
                                yt[:, kc, ts(mt, P)],
                                wp[:, kc, ts(nb, 512)],
                                start=(kc == 0),
                                stop=(kc == 1),
                            )
                        ot = op.tile([P, 512], f32, tag="ot")
                        nc.vector.tensor_copy(ot[:], pj[:])
                        nc.sync.dma_start(out_d.ap()[ts(mt, P), ts(nb, 512)], ot[:])

    nc.compile()
    return nc


def _get_nc():
    global _NC
    if _NC is None:
        _NC = _build()
    return _NC


def _shard(x, W_qkv, b_qkv, W_proj, b_proj):
    x = np.ascontiguousarray(np.asarray(x, dtype=np.float32))
    W_qkv = np.ascontiguousarray(np.asarray(W_qkv, dtype=np.float32))
    b_qkv = np.ascontiguousarray(np.asarray(b_qkv, dtype=np.float32))
    W_proj = np.ascontiguousarray(np.asarray(W_proj, dtype=np.float32))
    in_maps = []
    for core in range(8):
        b, g = core // 4, core % 4
        cs = slice(g * HC, (g + 1) * HC)
        in_maps.append(
            {
                "x": np.ascontiguousarray(x[b]),
                "wq": np.ascontiguousarray(W_qkv[:, 0 * C :][:, cs]),
                "wk": np.ascontiguousarray(W_qkv[:, 1 * C :][:, cs]),
                "wv": np.ascontiguousarray(W_qkv[:, 2 * C :][:, cs]),
                "wp": np.ascontiguousarray(W_proj[cs, :]),
                "bq": np.ascontiguousarray(b_qkv[0 * C :][cs]),
                "bk": np.ascontiguousarray(b_qkv[1 * C :][cs]),
                "bv": np.ascontiguousarray(b_qkv[2 * C :][cs]),
            }
        )
    return in_maps


def _gather(results, b_proj):
    b_proj = np.asarray(b_proj, dtype=np.float32)
    y = np.empty((2, T, C), dtype=np.float32)
    for b in range(2):
        acc = results[4 * b]["out"].astype(np.float32).copy()
        for g in range(1, 4):
            acc += results[4 * b + g]["out"]
        y[b] = acc + b_proj
    return y


def kernel(x, W_qkv, b_qkv, W_proj, b_proj):
    nc = _get_nc()
    in_maps = _shard(x, W_qkv, b_qkv, W_proj, b_proj)
    res = bass_utils.run_bass_kernel_spmd(nc, in_maps, core_ids=list(range(8)))
    return _gather(res.results, b_proj)


# revision 14
# speedup vs baseline: 1.1843x; 1.1843x over previous
"""Causal self-attention (B=2, T=2048, C=1024, H=16) on 8 TRN2 NeuronCores.

Sharding: data-parallel on batch (2) x tensor-parallel on heads (4 groups of
4 heads) = 8 cores. Each core computes, for its batch b and head group g:
  QKV^T projection for its 256 qkv columns, causal flash-style attention for
  its 4 heads, and a partial output projection  Y_g @ W_proj[256g:256(g+1)].
The host sums the 4 partial projections per batch and adds b_proj.

On-chip dataflow (all matmuls in float32r ~ tf32), fully pipelined per
512-query block so PE-bound projection work overlaps ACT-bound softmax:
  xT   = transpose(x_b)                [C-part, tq]     (PE transpose)
  Q^T  = Wq.T @ x via lhsT=Wq chunks   [qcol-part, tq]
  K^T  likewise; V natural             [tk-part, vcol]
  S^T  = K^T_blk.T @ Q^T               [tk-part, tq]  (2 heads row-packed)
  E    = exp(S^T/8) (ACT, PSUM->SBUF), triangle mask on diagonal strips
  Ynum^T, denom = [V_h | 1].T @ E      [65-part, tq]  (PSUM accumulated)
  Y^T  = Ynum^T * (1/denom)            (approx recip + GPSIMD bcast + DVE)
  out += Y^T.T @ Wp                    [tq-part, cout]
"""

import numpy as np

import concourse.bacc as bacc
import concourse.mybir as mybir
from concourse import bass_utils
from concourse.bass import ts
from concourse.masks import make_identity
from concourse.tile import TileContext

P = 128
T = 2048
C = 1024
KO = C // P          # 8 contraction chunks over C
HC = 256             # qkv columns per core (4 heads x 64)
NH = 4               # heads per core
D = 64
NTK = T // P         # 16 key blocks
TQB = 512            # query block (free dim)
NQ = T // TQB        # 4 query blocks
SCALE = 1.0 / np.sqrt(D)

f32 = mybir.dt.float32
f32r = mybir.dt.float32r
AF = mybir.ActivationFunctionType
ALU = mybir.AluOpType

_NC = None


def _build():
    nc = bacc.Bacc(trn_type="TRN2", target_bir_lowering=False, debug=False)

    x_d = nc.dram_tensor("x", [T, C], f32, kind="ExternalInput")
    wq_d = nc.dram_tensor("wq", [C, HC], f32r, kind="ExternalInput")
    wk_d = nc.dram_tensor("wk", [C, HC], f32r, kind="ExternalInput")
    wv_d = nc.dram_tensor("wv", [C, HC], f32r, kind="ExternalInput")
    wp_d = nc.dram_tensor("wp", [HC, C], f32r, kind="ExternalInput")
    bq_d = nc.dram_tensor("bq", [HC], f32, kind="ExternalInput")
    bk_d = nc.dram_tensor("bk", [HC], f32, kind="ExternalInput")
    bv_d = nc.dram_tensor("bv", [HC], f32, kind="ExternalInput")
    out_d = nc.dram_tensor("out", [T, C], f32, kind="ExternalOutput")

    with TileContext(nc) as tc:
        with (
            tc.tile_pool(name="persist", bufs=1) as pp,
            tc.tile_pool(name="psum", bufs=2, space="PSUM") as ps,
            tc.tile_pool(name="xs", bufs=2) as xsp,
            tc.tile_pool(name="xn", bufs=3) as xnp,
            tc.tile_pool(name="e", bufs=5) as ep,
            tc.tile_pool(name="r", bufs=4) as rp,
            tc.tile_pool(name="o", bufs=3) as op,
        ):
            wq = pp.tile([P, KO, HC], f32r, tag="wq")
            wk = pp.tile([P, KO, HC], f32r, tag="wk")
            wv = pp.tile([P, KO, HC], f32r, tag="wv")
            wp = pp.tile([P, 2, C], f32r, tag="wp")
            bqt = pp.tile([P, 2], f32, tag="bqt")
            bkt = pp.tile([P, 2], f32, tag="bkt")
            bvt = pp.tile([P, HC], f32, tag="bvt")
            qt = pp.tile([P, 2, T], f32r, tag="qt")
            kt = pp.tile([P, 2, T], f32r, tag="kt")
            v = pp.tile([P, NTK, NH, D + 1], f32r, tag="v")
            yt = pp.tile([P, 2, T], f32r, tag="yt")
            mask = pp.tile([P, P], f32, tag="mask")
            ident = pp.tile([P, P], f32, tag="ident")

            nc.sync.dma_start(wq[:], wq_d.ap().rearrange("(ko p) n -> p ko n", p=P))
            nc.sync.dma_start(wk[:], wk_d.ap().rearrange("(ko p) n -> p ko n", p=P))
            nc.sync.dma_start(wv[:], wv_d.ap().rearrange("(ko p) n -> p ko n", p=P))
            nc.sync.dma_start(wp[:], wp_d.ap().rearrange("(kc p) n -> p kc n", p=P))
            nc.sync.dma_start(bqt[:], bq_d.ap().rearrange("(c p) -> p c", p=P))
            nc.sync.dma_start(bkt[:], bk_d.ap().rearrange("(c p) -> p c", p=P))
            nc.sync.dma_start(bvt[:], bv_d.ap()[None, :].to_broadcast((P, HC)))

            make_identity(nc, ident[:])
            # ones columns for the denominator rows of V_aug
            nc.gpsimd.memset(v[:].bitcast(f32), 1.0)
            # triangle mask: mask[p, f] = 1 iff p <= f
            nc.gpsimd.memset(mask[:], 1.0)
            nc.gpsimd.affine_select(
                out=mask[:],
                in_=mask[:],
                compare_op=ALU.is_ge,
                fill=0.0,
                base=0,
                pattern=[[1, P]],
                channel_multiplier=-1,
            )
            tri = mask[:, 0:P].bitcast(f32r)

            for tqb in range(NQ):
                # -- transpose this 512-row slice of x into xs[C-part, 512] --
                xs = xsp.tile([P, KO, TQB], f32r, tag="xs")
                for lt in range(4):
                    ti = 4 * tqb + lt
                    xn = xnp.tile([P, C], f32, tag="xn")
                    nc.sync.dma_start(xn[:], x_d.ap()[ts(ti, P), :])
                    for kk in range(0, KO, 4):
                        pt = ps.tile([P, 512], f32, tag="mm512")
                        for j in range(4):
                            nc.tensor.transpose(
                                pt[:, ts(j, P)], xn[:, ts(kk + j, P)], ident[:]
                            )
                        nc.vector.tensor_copy(
                            xs[:, kk : kk + 4, ts(lt, P)],
                            pt[:].rearrange("p (k t) -> p k t", k=4),
                        )

                # -- Q^T / K^T for this query block --
                for cc in range(2):
                    pq = ps.tile([P, TQB], f32, tag="mm512")
                    for ko in range(KO):
                        nc.tensor.matmul(
                            pq[:],
                            wq[:, ko, ts(cc, P)],
                            xs[:, ko, :],
                            start=(ko == 0),
                            stop=(ko == KO - 1),
                        )
                    nc.vector.tensor_scalar_add(
                        qt[:, cc, ts(tqb, TQB)], pq[:], bqt[:, cc : cc + 1]
                    )
                    pk = ps.tile([P, TQB], f32, tag="mm512")
                    for ko in range(KO):
                        nc.tensor.matmul(
                            pk[:],
                            wk[:, ko, ts(cc, P)],
                            xs[:, ko, :],
                            start=(ko == 0),
                            stop=(ko == KO - 1),
                        )
                    nc.vector.tensor_scalar_add(
                        kt[:, cc, ts(tqb, TQB)], pk[:], bkt[:, cc : cc + 1]
                    )

                # -- V for the 4 key blocks of this slice --
                for lt in range(4):
                    tk = 4 * tqb + lt
                    pv = ps.tile([P, HC], f32, tag="mm512")
                    for ko in range(KO):
                        nc.tensor.matmul(
                            pv[:],
                            xs[:, ko, ts(lt, P)],
                            wv[:, ko, :],
                            start=(ko == 0),
                            stop=(ko == KO - 1),
                        )
                    nc.vector.tensor_tensor(
                        v[:, tk, :, 0:D],
                        pv[:].rearrange("p (h a) -> p h a", h=NH),
                        bvt[:].rearrange("p (h a) -> p h a", h=NH),
                        ALU.add,
                    )

                # -- causal attention for both head pairs --
                for hp in range(2):
                    h0, h1 = 2 * hp, 2 * hp + 1
                    y0 = ps.tile([D + 1, TQB], f32, tag="y")
                    y1 = ps.tile([D + 1, TQB], f32, tag="y")
                    q0 = qt[0:64, hp, ts(tqb, TQB)]
                    q1 = qt[64:128, hp, ts(tqb, TQB)]
                    ntk = 4 * (tqb + 1)
                    for tkk in range(0, ntk, 2):
                        s0 = ps.tile([P, 2 * TQB], f32, tag="s")
                        s1 = ps.tile([P, 2 * TQB], f32, tag="s")
                        for j in range(2):
                            tk = tkk + j
                            nc.tensor.matmul(
                                s0[:, ts(j, TQB)],
                                kt[0:64, hp, ts(tk, P)],
                                q0,
                                start=True,
                                stop=True,
                                tile_position=(0, 0),
                            )
                            nc.tensor.matmul(
                                s1[:, ts(j, TQB)],
                                kt[64:128, hp, ts(tk, P)],
                                q1,
                                start=True,
                                stop=True,
                                tile_position=(64, 0),
                            )
                        e0 = ep.tile([P, 2 * TQB], f32r, tag="e")
                        e1 = ep.tile([P, 2 * TQB], f32r, tag="e")
                        nc.scalar.activation(e0[:], s0[:], AF.Exp, scale=SCALE)
                        nc.scalar.activation(e1[:], s1[:], AF.Exp, scale=SCALE)
                        for j in range(2):
                            tk = tkk + j
                            jd = tk - 4 * tqb  # diagonal strip index
                            if jd >= 0:
                                # cols < 128*jd are fully masked; the
                                # [128*jd, 128*(jd+1)) strip is triangular
                                for e in (e0, e1):
                                    if jd > 0:
                                        nc.vector.memset(
                                            e[:, j * TQB : j * TQB + jd * P].bitcast(
                                                f32
                                            ),
                                            0.0,
                                        )
                                    st = j * TQB + jd * P
                                    nc.vector.tensor_mul(
                                        e[:, st : st + P], e[:, st : st + P], tri
                                    )
                        for j in range(2):
                            tk = tkk + j
                            nc.tensor.matmul(
                                y0[:],
                                v[:, tk, h0, :],
                                e0[:, ts(j, TQB)],
                                start=(tk == 0),
                                stop=(tk == ntk - 1),
                            )
                            nc.tensor.matmul(
                                y1[:],
                                v[:, tk, h1, :],
                                e1[:, ts(j, TQB)],
                                start=(tk == 0),
                                stop=(tk == ntk - 1),
                            )
                    den0 = rp.tile([1, TQB], f32, tag="den")
                    den1 = rp.tile([1, TQB], f32, tag="den")
                    nc.vector.tensor_copy(den0[:], y0[64:65, :])
                    nc.vector.tensor_copy(den1[:], y1[64:65, :])
                    rec0 = rp.tile([1, TQB], f32, tag="rec")
                    rec1 = rp.tile([1, TQB], f32, tag="rec")
                    nc.vector.reciprocal_approx_fast(rec0[:], den0[:])
                    nc.vector.reciprocal_approx_fast(rec1[:], den1[:])
                    rb0 = rp.tile([D, TQB], f32, tag="rb")
                    rb1 = rp.tile([D, TQB], f32, tag="rb")
                    nc.gpsimd.partition_broadcast(rb0[:], rec0[:])
                    nc.gpsimd.partition_broadcast(rb1[:], rec1[:])
                    nc.vector.tensor_mul(
                        yt[0:64, hp, ts(tqb, TQB)], y0[0:64, :], rb0[:]
                    )
                    nc.vector.tensor_mul(
                        yt[64:128, hp, ts(tqb, TQB)], y1[0:64, :], rb1[:]
                    )

                # -- partial output projection for this query block --
                for mt in range(4 * tqb, 4 * tqb + 4):
                    for nb in range(2):
                        pj = ps.tile([P, 512], f32, tag="y")
                        for kc in range(2):
                            nc.tensor.matmul(
                                pj[:],
                                yt[:, kc, ts(mt, P)],
                                wp[:, kc, ts(nb, 512)],
                                start=(kc == 0),
                                stop=(kc == 1),
                            )
                        ot = op.tile([P, 512], f32, tag="ot")
                        nc.vector.tensor_copy(ot[:], pj[:])
                        nc.sync.dma_start(out_d.ap()[ts(mt, P), ts(nb, 512)], ot[:])

    nc.compile()
    return nc


def _get_nc():
    global _NC
    if _NC is None:
        _NC = _build()
    return _NC


def _shard(x, W_qkv, b_qkv, W_proj, b_proj):
    x = np.ascontiguousarray(np.asarray(x, dtype=np.float32))
    W_qkv = np.ascontiguousarray(np.asarray(W_qkv, dtype=np.float32))
    b_qkv = np.ascontiguousarray(np.asarray(b_qkv, dtype=np.float32))
    W_proj = np.ascontiguousarray(np.asarray(W_proj, dtype=np.float32))
    in_maps = []
    for core in range(8):
        b, g = core // 4, core % 4
        cs = slice(g * HC, (g + 1) * HC)
        in_maps.append(
            {
                "x": np.ascontiguousarray(x[b]),
                "wq": np.ascontiguousarray(W_qkv[:, 0 * C :][:, cs]),
                "wk": np.ascontiguousarray(W_qkv[:, 1 * C :][:, cs]),
                "wv": np.ascontiguousarray(W_qkv[:, 2 * C :][:, cs]),
                "wp": np.ascontiguousarray(W_proj[cs, :]),
                "bq": np.ascontiguousarray(b_qkv[0 * C :][cs]),
                "bk": np.ascontiguousarray(b_qkv[1 * C :][cs]),
                "bv": np.ascontiguousarray(b_qkv[2 * C :][cs]),
            }
        )
    return in_maps


def _gather(results, b_proj):
    b_proj = np.asarray(b_proj, dtype=np.float32)
    y = np.empty((2, T, C), dtype=np.float32)
    for b in range(2):
        acc = results[4 * b]["out"].astype(np.float32).copy()
        for g in range(1, 4):
            acc += results[4 * b + g]["out"]
        y[b] = acc + b_proj
    return y


def kernel(x, W_qkv, b_qkv, W_proj, b_proj):
    nc = _get_nc()
    in_maps = _shard(x, W_qkv, b_qkv, W_proj, b_proj)
    res = bass_utils.run_bass_kernel_spmd(nc, in_maps, core_ids=list(range(8)))
    return _gather(res.results, b_proj)


# revision 16
# speedup vs baseline: 1.2007x; 1.0138x over previous
"""Causal self-attention (B=2, T=2048, C=1024, H=16) on 8 TRN2 NeuronCores.

Sharding: data-parallel on batch (2) x tensor-parallel on heads (4 groups of
4 heads) = 8 cores. Each core computes, for its batch b and head group g:
  QKV^T projection for its 256 qkv columns, causal flash-style attention for
  its 4 heads, and a partial output projection  Y_g @ W_proj[256g:256(g+1)].
The host sums the 4 partial projections per batch and adds b_proj.

On-chip dataflow (all matmuls in float32r ~ tf32), fully pipelined per
512-query block so PE-bound projection work overlaps ACT-bound softmax:
  xT   = transpose(x_b)                [C-part, tq]     (PE transpose)
  Q^T  = Wq.T @ x via lhsT=Wq chunks   [qcol-part, tq]
  K^T  likewise; V natural             [tk-part, vcol]
  S^T  = K^T_blk.T @ Q^T               [tk-part, tq]  (2 heads row-packed)
  E    = exp(S^T/8) (ACT, PSUM->SBUF), triangle mask on diagonal strips
  Ynum^T, denom = [V_h | 1].T @ E      [65-part, tq]  (PSUM accumulated)
  Y^T  = Ynum^T * (1/denom)            (approx recip + GPSIMD bcast + DVE)
  out += Y^T.T @ Wp                    [tq-part, cout]
"""

import ml_dtypes
import numpy as np

import concourse.bacc as bacc
import concourse.mybir as mybir
from concourse import bass_utils
from concourse.bass import ts
from concourse.masks import make_identity
from concourse.tile import TileContext

P = 128
T = 2048
C = 1024
KO = C // P          # 8 contraction chunks over C
HC = 256             # qkv columns per core (4 heads x 64)
NH = 4               # heads per core
D = 64
NTK = T // P         # 16 key blocks
TQB = 512            # query block (free dim)
NQ = T // TQB        # 4 query blocks
SCALE = 1.0 / np.sqrt(D)

f32 = mybir.dt.float32
bf16 = mybir.dt.bfloat16
AF = mybir.ActivationFunctionType
ALU = mybir.AluOpType

_NC = None


def _build():
    nc = bacc.Bacc(trn_type="TRN2", target_bir_lowering=False, debug=False)

    x_d = nc.dram_tensor("x", [T, C], f32, kind="ExternalInput")
    wq_d = nc.dram_tensor("wq", [C, HC], bf16, kind="ExternalInput")
    wk_d = nc.dram_tensor("wk", [C, HC], bf16, kind="ExternalInput")
    wv_d = nc.dram_tensor("wv", [C, HC], bf16, kind="ExternalInput")
    wp_d = nc.dram_tensor("wp", [HC, C], bf16, kind="ExternalInput")
    bq_d = nc.dram_tensor("bq", [HC], f32, kind="ExternalInput")
    bk_d = nc.dram_tensor("bk", [HC], f32, kind="ExternalInput")
    bv_d = nc.dram_tensor("bv", [HC], f32, kind="ExternalInput")
    out_d = nc.dram_tensor("out", [T, C], f32, kind="ExternalOutput")

    with TileContext(nc) as tc:
        with (
            tc.tile_pool(name="persist", bufs=1) as pp,
            tc.tile_pool(name="psum", bufs=2, space="PSUM") as ps,
            tc.tile_pool(name="xs", bufs=2) as xsp,
            tc.tile_pool(name="xn", bufs=3) as xnp,
            tc.tile_pool(name="e", bufs=5) as ep,
            tc.tile_pool(name="r", bufs=4) as rp,
            tc.tile_pool(name="o", bufs=3) as op,
        ):
            wq = pp.tile([P, KO, HC], bf16, tag="wq")
            wk = pp.tile([P, KO, HC], bf16, tag="wk")
            wv = pp.tile([P, KO, HC], bf16, tag="wv")
            wp = pp.tile([P, 2, C], bf16, tag="wp")
            bqt = pp.tile([P, 2], f32, tag="bqt")
            bkt = pp.tile([P, 2], f32, tag="bkt")
            bvt = pp.tile([P, HC], f32, tag="bvt")
            qt = pp.tile([P, 2, T], bf16, tag="qt")
            kt = pp.tile([P, 2, T], bf16, tag="kt")
            v = pp.tile([P, NTK, NH, D + 1], bf16, tag="v")
            yt = pp.tile([P, 2, T], bf16, tag="yt")
            mask = pp.tile([P, P], bf16, tag="mask")
            ident = pp.tile([P, P], bf16, tag="ident")

            nc.sync.dma_start(wq[:], wq_d.ap().rearrange("(ko p) n -> p ko n", p=P))
            nc.sync.dma_start(wk[:], wk_d.ap().rearrange("(ko p) n -> p ko n", p=P))
            nc.sync.dma_start(wv[:], wv_d.ap().rearrange("(ko p) n -> p ko n", p=P))
            nc.sync.dma_start(wp[:], wp_d.ap().rearrange("(kc p) n -> p kc n", p=P))
            nc.sync.dma_start(bqt[:], bq_d.ap().rearrange("(c p) -> p c", p=P))
            nc.sync.dma_start(bkt[:], bk_d.ap().rearrange("(c p) -> p c", p=P))
            nc.sync.dma_start(bvt[:], bv_d.ap()[None, :].to_broadcast((P, HC)))

            make_identity(nc, ident[:])
            # ones columns for the denominator rows of V_aug
            nc.gpsimd.memset(v[:], 1.0)
            # triangle mask: mask[p, f] = 1 iff p <= f
            nc.gpsimd.memset(mask[:], 1.0)
            nc.gpsimd.affine_select(
                out=mask[:],
                in_=mask[:],
                compare_op=ALU.is_ge,
                fill=0.0,
                base=0,
                pattern=[[1, P]],
                channel_multiplier=-1,
            )
            tri = mask[:, 0:P]

            for tqb in range(NQ):
                # -- transpose this 512-row slice of x into xs[C-part, 512] --
                xs = xsp.tile([P, KO, TQB], bf16, tag="xs")
                for lt in range(4):
                    ti = 4 * tqb + lt
                    xn = xnp.tile([P, C], f32, tag="xn")
                    nc.sync.dma_start(xn[:], x_d.ap()[ts(ti, P), :])
                    xb = xnp.tile([P, C], bf16, tag="xb")
                    nc.gpsimd.tensor_copy(xb[:], xn[:])
                    for kk in range(0, KO, 4):
                        pt = ps.tile([P, 512], bf16, tag="mm512")
                        for j in range(4):
                            nc.tensor.transpose(
                                pt[:, ts(j, P)], xb[:, ts(kk + j, P)], ident[:]
                            )
                        nc.vector.tensor_copy(
                            xs[:, kk : kk + 4, ts(lt, P)],
                            pt[:].rearrange("p (k t) -> p k t", k=4),
                        )

                # -- Q^T / K^T for this query block --
                for cc in range(2):
                    pq = ps.tile([P, TQB], f32, tag="mm512")
                    for ko in range(KO):
                        nc.tensor.matmul(
                            pq[:],
                            wq[:, ko, ts(cc, P)],
                            xs[:, ko, :],
                            start=(ko == 0),
                            stop=(ko == KO - 1),
                        )
                    nc.vector.tensor_scalar_add(
                        qt[:, cc, ts(tqb, TQB)], pq[:], bqt[:, cc : cc + 1]
                    )
                    pk = ps.tile([P, TQB], f32, tag="mm512")
                    for ko in range(KO):
                        nc.tensor.matmul(
                            pk[:],
                            wk[:, ko, ts(cc, P)],
                            xs[:, ko, :],
                            start=(ko == 0),
                            stop=(ko == KO - 1),
                        )
                    nc.vector.tensor_scalar_add(
                        kt[:, cc, ts(tqb, TQB)], pk[:], bkt[:, cc : cc + 1]
                    )

                # -- V for the 4 key blocks of this slice --
                for lt in range(4):
                    tk = 4 * tqb + lt
                    pv = ps.tile([P, HC], f32, tag="mm512")
                    for ko in range(KO):
                        nc.tensor.matmul(
                            pv[:],
                            xs[:, ko, ts(lt, P)],
                            wv[:, ko, :],
                            start=(ko == 0),
                            stop=(ko == KO - 1),
                        )
                    nc.vector.tensor_tensor(
                        v[:, tk, :, 0:D],
                        pv[:].rearrange("p (h a) -> p h a", h=NH),
                        bvt[:].rearrange("p (h a) -> p h a", h=NH),
                        ALU.add,
                    )

                # -- causal attention for both head pairs --
                for hp in range(2):
                    h0, h1 = 2 * hp, 2 * hp + 1
                    y0 = ps.tile([D + 1, TQB], f32, tag="y")
                    y1 = ps.tile([D + 1, TQB], f32, tag="y")
                    q0 = qt[0:64, hp, ts(tqb, TQB)]
                    q1 = qt[64:128, hp, ts(tqb, TQB)]
                    ntk = 4 * (tqb + 1)
                    for tkk in range(0, ntk, 2):
                        s0 = ps.tile([P, 2 * TQB], f32, tag="s")
                        s1 = ps.tile([P, 2 * TQB], f32, tag="s")
                        for j in range(2):
                            tk = tkk + j
                            nc.tensor.matmul(
                                s0[:, ts(j, TQB)],
                                kt[0:64, hp, ts(tk, P)],
                                q0,
                                start=True,
                                stop=True,
                                tile_position=(0, 0),
                            )
                            nc.tensor.matmul(
                                s1[:, ts(j, TQB)],
                                kt[64:128, hp, ts(tk, P)],
                                q1,
                                start=True,
                                stop=True,
                                tile_position=(64, 0),
                            )
                        e0 = ep.tile([P, 2 * TQB], bf16, tag="e")
                        e1 = ep.tile([P, 2 * TQB], bf16, tag="e")
                        nc.scalar.activation(e0[:], s0[:], AF.Exp, scale=SCALE)
                        nc.scalar.activation(e1[:], s1[:], AF.Exp, scale=SCALE)
                        for j in range(2):
                            tk = tkk + j
                            jd = tk - 4 * tqb  # diagonal strip index
                            if jd >= 0:
                                # cols < 128*jd are fully masked; the
                                # [128*jd, 128*(jd+1)) strip is triangular
                                for e in (e0, e1):
                                    if jd > 0:
                                        nc.vector.memset(
                                            e[:, j * TQB : j * TQB + jd * P], 0.0
                                        )
                                    st = j * TQB + jd * P
                                    nc.vector.tensor_mul(
                                        e[:, st : st + P], e[:, st : st + P], tri
                                    )
                        for j in range(2):
                            tk = tkk + j
                            nc.tensor.matmul(
                                y0[:],
                                v[:, tk, h0, :],
                                e0[:, ts(j, TQB)],
                                start=(tk == 0),
                                stop=(tk == ntk - 1),
                            )
                            nc.tensor.matmul(
                                y1[:],
                                v[:, tk, h1, :],
                                e1[:, ts(j, TQB)],
                                start=(tk == 0),
                                stop=(tk == ntk - 1),
                            )
                    den0 = rp.tile([1, TQB], f32, tag="den")
                    den1 = rp.tile([1, TQB], f32, tag="den")
                    nc.vector.tensor_copy(den0[:], y0[64:65, :])
                    nc.vector.tensor_copy(den1[:], y1[64:65, :])
                    rec0 = rp.tile([1, TQB], f32, tag="rec")
                    rec1 = rp.tile([1, TQB], f32, tag="rec")
                    nc.vector.reciprocal_approx_fast(rec0[:], den0[:])
                    nc.vector.reciprocal_approx_fast(rec1[:], den1[:])
                    rb0 = rp.tile([D, TQB], f32, tag="rb")
                    rb1 = rp.tile([D, TQB], f32, tag="rb")
                    nc.gpsimd.partition_broadcast(rb0[:], rec0[:])
                    nc.gpsimd.partition_broadcast(rb1[:], rec1[:])
                    nc.vector.tensor_mul(
                        yt[0:64, hp, ts(tqb, TQB)], y0[0:64, :], rb0[:]
                    )
                    nc.vector.tensor_mul(
                        yt[64:128, hp, ts(tqb, TQB)], y1[0:64, :], rb1[:]
                    )

                # -- partial output projection for this query block --
                for mt in range(4 * tqb, 4 * tqb + 4):
                    for nb in range(2):
                        pj = ps.tile([P, 512], f32, tag="y")
                        for kc in range(2):
                            nc.tensor.matmul(
                                pj[:],
                                yt[:, kc, ts(mt, P)],
                                wp[:, kc, ts(nb, 512)],
                                start=(kc == 0),
                                stop=(kc == 1),
                            )
                        ot = op.tile([P, 512], f32, tag="ot")
                        nc.vector.tensor_copy(ot[:], pj[:])
                        nc.sync.dma_start(out_d.ap()[ts(mt, P), ts(nb, 512)], ot[:])

    nc.compile()
    return nc


def _get_nc():
    global _NC
    if _NC is None:
        _NC = _build()
    return _NC


def _shard(x, W_qkv, b_qkv, W_proj, b_proj):
    x = np.ascontiguousarray(np.asarray(x, dtype=np.float32))
    W_qkv = np.ascontiguousarray(np.asarray(W_qkv, dtype=np.float32))
    b_qkv = np.ascontiguousarray(np.asarray(b_qkv, dtype=np.float32))
    W_proj = np.ascontiguousarray(np.asarray(W_proj, dtype=np.float32))
    in_maps = []
    for core in range(8):
        b, g = core // 4, core % 4
        cs = slice(g * HC, (g + 1) * HC)
        in_maps.append(
            {
                "x": np.ascontiguousarray(x[b]),
                "wq": np.ascontiguousarray(
                    W_qkv[:, 0 * C :][:, cs].astype(ml_dtypes.bfloat16)
                ),
                "wk": np.ascontiguousarray(
                    W_qkv[:, 1 * C :][:, cs].astype(ml_dtypes.bfloat16)
                ),
                "wv": np.ascontiguousarray(
                    W_qkv[:, 2 * C :][:, cs].astype(ml_dtypes.bfloat16)
                ),
                "wp": np.ascontiguousarray(W_proj[cs, :].astype(ml_dtypes.bfloat16)),
                "bq": np.ascontiguousarray(b_qkv[0 * C :][cs]),
                "bk": np.ascontiguousarray(b_qkv[1 * C :][cs]),
                "bv": np.ascontiguousarray(b_qkv[2 * C :][cs]),
            }
        )
    return in_maps


def _gather(results, b_proj):
    b_proj = np.asarray(b_proj, dtype=np.float32)
    y = np.empty((2, T, C), dtype=np.float32)
    for b in range(2):
        acc = results[4 * b]["out"].astype(np.float32).copy()
        for g in range(1, 4):
            acc += results[4 * b + g]["out"]
        y[b] = acc + b_proj
    return y


def kernel(x, W_qkv, b_qkv, W_proj, b_proj):
    nc = _get_nc()
    in_maps = _shard(x, W_qkv, b_qkv, W_proj, b_proj)
    res = bass_utils.run_bass_kernel_spmd(nc, in_maps, core_ids=list(range(8)))
    return _gather(res.results, b_proj)


# revision 17
# speedup vs baseline: 1.3098x; 1.0908x over previous
"""Causal self-attention (B=2, T=2048, C=1024, H=16) on 8 TRN2 NeuronCores.

Sharding: data-parallel on batch (2) x tensor-parallel on heads (4 groups of
4 heads) = 8 cores. Each core computes, for its batch b and head group g:
  QKV^T projection for its 256 qkv columns, causal flash-style attention for
  its 4 heads, and a partial output projection  Y_g @ W_proj[256g:256(g+1)].
The host sums the 4 partial projections per batch and adds b_proj.

On-chip dataflow (all matmuls in float32r ~ tf32), fully pipelined per
512-query block so PE-bound projection work overlaps ACT-bound softmax:
  xT   = transpose(x_b)                [C-part, tq]     (PE transpose)
  Q^T  = Wq.T @ x via lhsT=Wq chunks   [qcol-part, tq]
  K^T  likewise; V natural             [tk-part, vcol]
  S^T  = K^T_blk.T @ Q^T               [tk-part, tq]  (2 heads row-packed)
  E    = exp(S^T/8) (ACT, PSUM->SBUF), triangle mask on diagonal strips
  Ynum^T, denom = [V_h | 1].T @ E      [65-part, tq]  (PSUM accumulated)
  Y^T  = Ynum^T * (1/denom)            (approx recip + GPSIMD bcast + DVE)
  out += Y^T.T @ Wp                    [tq-part, cout]
"""

import ml_dtypes
import numpy as np

import concourse.bacc as bacc
import concourse.mybir as mybir
from concourse import bass_utils
from concourse.bass import ts
from concourse.masks import make_identity
from concourse.tile import TileContext

P = 128
T = 2048
C = 1024
KO = C // P          # 8 contraction chunks over C
HC = 256             # qkv columns per core (4 heads x 64)
NH = 4               # heads per core
D = 64
NTK = T // P         # 16 key blocks
TQB = 512            # query block (free dim)
NQ = T // TQB        # 4 query blocks
SCALE = 1.0 / np.sqrt(D)

f32 = mybir.dt.float32
bf16 = mybir.dt.bfloat16
AF = mybir.ActivationFunctionType
ALU = mybir.AluOpType

_NC = None


def _build():
    nc = bacc.Bacc(trn_type="TRN2", target_bir_lowering=False, debug=False)

    x_d = nc.dram_tensor("x", [T, C], f32, kind="ExternalInput")
    wq_d = nc.dram_tensor("wq", [C, HC], bf16, kind="ExternalInput")
    wk_d = nc.dram_tensor("wk", [C, HC], bf16, kind="ExternalInput")
    wv_d = nc.dram_tensor("wv", [C, HC], bf16, kind="ExternalInput")
    wp_d = nc.dram_tensor("wp", [HC, C], bf16, kind="ExternalInput")
    bq_d = nc.dram_tensor("bq", [HC], f32, kind="ExternalInput")
    bk_d = nc.dram_tensor("bk", [HC], f32, kind="ExternalInput")
    bv_d = nc.dram_tensor("bv", [HC], f32, kind="ExternalInput")
    out_d = nc.dram_tensor("out", [T, C], f32, kind="ExternalOutput")

    with TileContext(nc) as tc:
        with (
            tc.tile_pool(name="persist", bufs=1) as pp,
            tc.tile_pool(name="psum", bufs=2, space="PSUM") as ps,
            tc.tile_pool(name="xs", bufs=3) as xsp,
            tc.tile_pool(name="xn", bufs=4) as xnp,
            tc.tile_pool(name="e", bufs=6) as ep,
            tc.tile_pool(name="r", bufs=4) as rp,
            tc.tile_pool(name="o", bufs=3) as op,
        ):
            wq = pp.tile([P, KO, HC], bf16, tag="wq")
            wk = pp.tile([P, KO, HC], bf16, tag="wk")
            wv = pp.tile([P, KO, HC], bf16, tag="wv")
            wp = pp.tile([P, 2, C], bf16, tag="wp")
            bqt = pp.tile([P, 2], f32, tag="bqt")
            bkt = pp.tile([P, 2], f32, tag="bkt")
            bvt = pp.tile([P, HC], f32, tag="bvt")
            qt = pp.tile([P, 2, T], bf16, tag="qt")
            kt = pp.tile([P, 2, T], bf16, tag="kt")
            v = pp.tile([P, NTK, NH, D + 1], bf16, tag="v")
            yt = pp.tile([P, 2, T], bf16, tag="yt")
            mask = pp.tile([P, P], bf16, tag="mask")
            ident = pp.tile([P, P], bf16, tag="ident")

            nc.sync.dma_start(wq[:], wq_d.ap().rearrange("(ko p) n -> p ko n", p=P))
            nc.sync.dma_start(wk[:], wk_d.ap().rearrange("(ko p) n -> p ko n", p=P))
            nc.sync.dma_start(wv[:], wv_d.ap().rearrange("(ko p) n -> p ko n", p=P))
            nc.sync.dma_start(wp[:], wp_d.ap().rearrange("(kc p) n -> p kc n", p=P))
            nc.sync.dma_start(bqt[:], bq_d.ap().rearrange("(c p) -> p c", p=P))
            nc.sync.dma_start(bkt[:], bk_d.ap().rearrange("(c p) -> p c", p=P))
            nc.sync.dma_start(bvt[:], bv_d.ap()[None, :].to_broadcast((P, HC)))

            make_identity(nc, ident[:])
            # ones columns for the denominator rows of V_aug
            nc.gpsimd.memset(v[:], 1.0)
            # triangle mask: mask[p, f] = 1 iff p <= f
            nc.gpsimd.memset(mask[:], 1.0)
            nc.gpsimd.affine_select(
                out=mask[:],
                in_=mask[:],
                compare_op=ALU.is_ge,
                fill=0.0,
                base=0,
                pattern=[[1, P]],
                channel_multiplier=-1,
            )
            tri = mask[:, 0:P]

            for tqb in range(NQ):
                # -- transpose this 512-row slice of x into xs[C-part, 512] --
                xs = xsp.tile([P, KO, TQB], bf16, tag="xs")
                for lt in range(4):
                    ti = 4 * tqb + lt
                    xn = xnp.tile([P, C], f32, tag="xn")
                    nc.sync.dma_start(xn[:], x_d.ap()[ts(ti, P), :])
                    xb = xnp.tile([P, C], bf16, tag="xb")
                    nc.vector.tensor_copy(xb[:], xn[:])
                    for kk in range(0, KO, 4):
                        pt = ps.tile([P, 512], bf16, tag="mm512")
                        for j in range(4):
                            nc.tensor.transpose(
                                pt[:, ts(j, P)], xb[:, ts(kk + j, P)], ident[:]
                            )
                        nc.vector.tensor_copy(
                            xs[:, kk : kk + 4, ts(lt, P)],
                            pt[:].rearrange("p (k t) -> p k t", k=4),
                        )

                # -- Q^T / K^T for this query block --
                for cc in range(2):
                    pq = ps.tile([P, TQB], f32, tag="mm512")
                    for ko in range(KO):
                        nc.tensor.matmul(
                            pq[:],
                            wq[:, ko, ts(cc, P)],
                            xs[:, ko, :],
                            start=(ko == 0),
                            stop=(ko == KO - 1),
                        )
                    nc.vector.tensor_scalar_add(
                        qt[:, cc, ts(tqb, TQB)], pq[:], bqt[:, cc : cc + 1]
                    )
                    pk = ps.tile([P, TQB], f32, tag="mm512")
                    for ko in range(KO):
                        nc.tensor.matmul(
                            pk[:],
                            wk[:, ko, ts(cc, P)],
                            xs[:, ko, :],
                            start=(ko == 0),
                            stop=(ko == KO - 1),
                        )
                    nc.vector.tensor_scalar_add(
                        kt[:, cc, ts(tqb, TQB)], pk[:], bkt[:, cc : cc + 1]
                    )

                # -- V for the 4 key blocks of this slice --
                for lt in range(4):
                    tk = 4 * tqb + lt
                    pv = ps.tile([P, HC], f32, tag="mm512")
                    for ko in range(KO):
                        nc.tensor.matmul(
                            pv[:],
                            xs[:, ko, ts(lt, P)],
                            wv[:, ko, :],
                            start=(ko == 0),
                            stop=(ko == KO - 1),
                        )
                    nc.vector.tensor_tensor(
                        v[:, tk, :, 0:D],
                        pv[:].rearrange("p (h a) -> p h a", h=NH),
                        bvt[:].rearrange("p (h a) -> p h a", h=NH),
                        ALU.add,
                    )

                # -- causal attention for both head pairs --
                for hp in range(2):
                    h0, h1 = 2 * hp, 2 * hp + 1
                    y0 = ps.tile([D + 1, TQB], f32, tag="y")
                    y1 = ps.tile([D + 1, TQB], f32, tag="y")
                    q0 = qt[0:64, hp, ts(tqb, TQB)]
                    q1 = qt[64:128, hp, ts(tqb, TQB)]
                    ntk = 4 * (tqb + 1)
                    for tkk in range(0, ntk, 2):
                        s0 = ps.tile([P, 2 * TQB], f32, tag="s")
                        s1 = ps.tile([P, 2 * TQB], f32, tag="s")
                        for j in range(2):
                            tk = tkk + j
                            nc.tensor.matmul(
                                s0[:, ts(j, TQB)],
                                kt[0:64, hp, ts(tk, P)],
                                q0,
                                start=True,
                                stop=True,
                                tile_position=(0, 0),
                            )
                            nc.tensor.matmul(
                                s1[:, ts(j, TQB)],
                                kt[64:128, hp, ts(tk, P)],
                                q1,
                                start=True,
                                stop=True,
                                tile_position=(64, 0),
                            )
                        e0 = ep.tile([P, 2 * TQB], bf16, tag="e")
                        e1 = ep.tile([P, 2 * TQB], bf16, tag="e")
                        nc.scalar.activation(e0[:], s0[:], AF.Exp, scale=SCALE)
                        nc.scalar.activation(e1[:], s1[:], AF.Exp, scale=SCALE)
                        for j in range(2):
                            tk = tkk + j
                            jd = tk - 4 * tqb  # diagonal strip index
                            if jd >= 0:
                                # cols < 128*jd are fully masked; the
                                # [128*jd, 128*(jd+1)) strip is triangular
                                for e in (e0, e1):
                                    if jd > 0:
                                        nc.vector.memset(
                                            e[:, j * TQB : j * TQB + jd * P], 0.0
                                        )
                                    st = j * TQB + jd * P
                                    nc.vector.tensor_mul(
                                        e[:, st : st + P], e[:, st : st + P], tri
                                    )
                        for j in range(2):
                            tk = tkk + j
                            nc.tensor.matmul(
                                y0[:],
                                v[:, tk, h0, :],
                                e0[:, ts(j, TQB)],
                                start=(tk == 0),
                                stop=(tk == ntk - 1),
                            )
                            nc.tensor.matmul(
                                y1[:],
                                v[:, tk, h1, :],
                                e1[:, ts(j, TQB)],
                                start=(tk == 0),
                                stop=(tk == ntk - 1),
                            )
                    den0 = rp.tile([1, TQB], f32, tag="den")
                    den1 = rp.tile([1, TQB], f32, tag="den")
                    nc.vector.tensor_copy(den0[:], y0[64:65, :])
                    nc.vector.tensor_copy(den1[:], y1[64:65, :])
                    rec0 = rp.tile([1, TQB], f32, tag="rec")
                    rec1 = rp.tile([1, TQB], f32, tag="rec")
                    nc.vector.reciprocal_approx_fast(rec0[:], den0[:])
                    nc.vector.reciprocal_approx_fast(rec1[:], den1[:])
                    rb0 = rp.tile([D, TQB], f32, tag="rb")
                    rb1 = rp.tile([D, TQB], f32, tag="rb")
                    nc.gpsimd.partition_broadcast(rb0[:], rec0[:])
                    nc.gpsimd.partition_broadcast(rb1[:], rec1[:])
                    nc.vector.tensor_mul(
                        yt[0:64, hp, ts(tqb, TQB)], y0[0:64, :], rb0[:]
                    )
                    nc.vector.tensor_mul(
                        yt[64:128, hp, ts(tqb, TQB)], y1[0:64, :], rb1[:]
                    )

                # -- partial output projection for this query block --
                for mt in range(4 * tqb, 4 * tqb + 4):
                    for nb in range(2):
                        pj = ps.tile([P, 512], f32, tag="y")
                        for kc in range(2):
                            nc.tensor.matmul(
                                pj[:],
                                yt[:, kc, ts(mt, P)],
                                wp[:, kc, ts(nb, 512)],
                                start=(kc == 0),
                                stop=(kc == 1),
                            )
                        ot = op.tile([P, 512], f32, tag="ot")
                        nc.vector.tensor_copy(ot[:], pj[:])
                        nc.sync.dma_start(out_d.ap()[ts(mt, P), ts(nb, 512)], ot[:])

    nc.compile()
    return nc


def _get_nc():
    global _NC
    if _NC is None:
        _NC = _build()
    return _NC


def _shard(x, W_qkv, b_qkv, W_proj, b_proj):
    x = np.ascontiguousarray(np.asarray(x, dtype=np.float32))
    W_qkv = np.ascontiguousarray(np.asarray(W_qkv, dtype=np.float32))
    b_qkv = np.ascontiguousarray(np.asarray(b_qkv, dtype=np.float32))
    W_proj = np.ascontiguousarray(np.asarray(W_proj, dtype=np.float32))
    in_maps = []
    for core in range(8):
        b, g = core // 4, core % 4
        cs = slice(g * HC, (g + 1) * HC)
        in_maps.append(
            {
                "x": np.ascontiguousarray(x[b]),
                "wq": np.ascontiguousarray(
                    W_qkv[:, 0 * C :][:, cs].astype(ml_dtypes.bfloat16)
                ),
                "wk": np.ascontiguousarray(
                    W_qkv[:, 1 * C :][:, cs].astype(ml_dtypes.bfloat16)
                ),
                "wv": np.ascontiguousarray(
                    W_qkv[:, 2 * C :][:, cs].astype(ml_dtypes.bfloat16)
                ),
                "wp": np.ascontiguousarray(W_proj[cs, :].astype(ml_dtypes.bfloat16)),
                "bq": np.ascontiguousarray(b_qkv[0 * C :][cs]),
                "bk": np.ascontiguousarray(b_qkv[1 * C :][cs]),
                "bv": np.ascontiguousarray(b_qkv[2 * C :][cs]),
            }
        )
    return in_maps


def _gather(results, b_proj):
    b_proj = np.asarray(b_proj, dtype=np.float32)
    y = np.empty((2, T, C), dtype=np.float32)
    for b in range(2):
        acc = results[4 * b]["out"].astype(np.float32).copy()
        for g in range(1, 4):
            acc += results[4 * b + g]["out"]
        y[b] = acc + b_proj
    return y


def kernel(x, W_qkv, b_qkv, W_proj, b_proj):
    nc = _get_nc()
    in_maps = _shard(x, W_qkv, b_qkv, W_proj, b_proj)
    res = bass_utils.run_bass_kernel_spmd(nc, in_maps, core_ids=list(range(8)))
    return _gather(res.results, b_proj)


# revision 19
# speedup vs baseline: 1.3317x; 1.0167x over previous
"""Causal self-attention (B=2, T=2048, C=1024, H=16) on 8 TRN2 NeuronCores.

Sharding: data-parallel on batch (2) x tensor-parallel on heads (4 groups of
4 heads) = 8 cores. Each core computes, for its batch b and head group g:
  QKV^T projection for its 256 qkv columns, causal flash-style attention for
  its 4 heads, and a partial output projection  Y_g @ W_proj[256g:256(g+1)].
The host sums the 4 partial projections per batch and adds b_proj.

On-chip dataflow (all matmuls in float32r ~ tf32), fully pipelined per
512-query block so PE-bound projection work overlaps ACT-bound softmax:
  xT   = transpose(x_b)                [C-part, tq]     (PE transpose)
  Q^T  = Wq.T @ x via lhsT=Wq chunks   [qcol-part, tq]
  K^T  likewise; V natural             [tk-part, vcol]
  S^T  = K^T_blk.T @ Q^T               [tk-part, tq]  (2 heads row-packed)
  E    = exp(S^T/8) (ACT, PSUM->SBUF), triangle mask on diagonal strips
  Ynum^T, denom = [V_h | 1].T @ E      [65-part, tq]  (PSUM accumulated)
  Y^T  = Ynum^T * (1/denom)            (approx recip + GPSIMD bcast + DVE)
  out += Y^T.T @ Wp                    [tq-part, cout]
"""

import ml_dtypes
import numpy as np

import concourse.bacc as bacc
import concourse.mybir as mybir
from concourse import bass_utils
from concourse.bass import ts
from concourse.masks import make_identity
from concourse.tile import TileContext

P = 128
T = 2048
C = 1024
KO = C // P          # 8 contraction chunks over C
HC = 256             # qkv columns per core (4 heads x 64)
NH = 4               # heads per core
D = 64
NTK = T // P         # 16 key blocks
TQB = 512            # query block (free dim)
NQ = T // TQB        # 4 query blocks
SCALE = 1.0 / np.sqrt(D)

f32 = mybir.dt.float32
bf16 = mybir.dt.bfloat16
AF = mybir.ActivationFunctionType
ALU = mybir.AluOpType

_NC = None


def _build():
    nc = bacc.Bacc(trn_type="TRN2", target_bir_lowering=False, debug=False)

    x_d = nc.dram_tensor("x", [T, C], f32, kind="ExternalInput")
    wq_d = nc.dram_tensor("wq", [C, HC], bf16, kind="ExternalInput")
    wk_d = nc.dram_tensor("wk", [C, HC], bf16, kind="ExternalInput")
    wv_d = nc.dram_tensor("wv", [C, HC], bf16, kind="ExternalInput")
    wp_d = nc.dram_tensor("wp", [HC, C], bf16, kind="ExternalInput")
    bq_d = nc.dram_tensor("bq", [HC], f32, kind="ExternalInput")
    bk_d = nc.dram_tensor("bk", [HC], f32, kind="ExternalInput")
    bv_d = nc.dram_tensor("bv", [HC], f32, kind="ExternalInput")
    out_d = nc.dram_tensor("out", [T, C], f32, kind="ExternalOutput")

    with TileContext(nc) as tc:
        with (
            tc.tile_pool(name="persist", bufs=1) as pp,
            tc.tile_pool(name="psum", bufs=2, space="PSUM") as ps,
            tc.tile_pool(name="xs", bufs=3) as xsp,
            tc.tile_pool(name="xn", bufs=4) as xnp,
            tc.tile_pool(name="e", bufs=6) as ep,
            tc.tile_pool(name="r", bufs=4) as rp,
            tc.tile_pool(name="o", bufs=3) as op,
        ):
            wq = pp.tile([P, KO, HC], bf16, tag="wq")
            wk = pp.tile([P, KO, HC], bf16, tag="wk")
            wv = pp.tile([P, KO, HC], bf16, tag="wv")
            wp = pp.tile([P, 2, C], bf16, tag="wp")
            bqt = pp.tile([P, 2], f32, tag="bqt")
            bkt = pp.tile([P, 2], f32, tag="bkt")
            bvt = pp.tile([P, HC], f32, tag="bvt")
            qt = pp.tile([P, 2, T], bf16, tag="qt")
            kt = pp.tile([P, 2, T], bf16, tag="kt")
            v = pp.tile([P, NTK, NH, D + 1], bf16, tag="v")
            yt = pp.tile([P, 2, T], bf16, tag="yt")
            mask = pp.tile([P, P], bf16, tag="mask")
            ident = pp.tile([P, P], bf16, tag="ident")

            nc.sync.dma_start(wq[:], wq_d.ap().rearrange("(ko p) n -> p ko n", p=P))
            nc.sync.dma_start(wk[:], wk_d.ap().rearrange("(ko p) n -> p ko n", p=P))
            nc.sync.dma_start(wv[:], wv_d.ap().rearrange("(ko p) n -> p ko n", p=P))
            nc.sync.dma_start(wp[:], wp_d.ap().rearrange("(kc p) n -> p kc n", p=P))
            nc.sync.dma_start(bqt[:], bq_d.ap().rearrange("(c p) -> p c", p=P))
            nc.sync.dma_start(bkt[:], bk_d.ap().rearrange("(c p) -> p c", p=P))
            nc.sync.dma_start(bvt[:], bv_d.ap()[None, :].to_broadcast((P, HC)))

            make_identity(nc, ident[:])
            # ones columns for the denominator rows of V_aug
            nc.gpsimd.memset(v[:], 1.0)
            # triangle mask: mask[p, f] = 1 iff p <= f
            nc.gpsimd.memset(mask[:], 1.0)
            nc.gpsimd.affine_select(
                out=mask[:],
                in_=mask[:],
                compare_op=ALU.is_ge,
                fill=0.0,
                base=0,
                pattern=[[1, P]],
                channel_multiplier=-1,
            )
            tri = mask[:, 0:P]

            def emit_proj(mt, nb):
                pj = ps.tile([P, 512], f32, tag="y")
                for kc in range(2):
                    nc.tensor.matmul(
                        pj[:],
                        yt[:, kc, ts(mt, P)],
                        wp[:, kc, ts(nb, 512)],
                        start=(kc == 0),
                        stop=(kc == 1),
                    )
                ot = op.tile([P, 512], f32, tag="ot")
                nc.vector.tensor_copy(ot[:], pj[:])
                nc.sync.dma_start(out_d.ap()[ts(mt, P), ts(nb, 512)], ot[:])

            pending_proj = []
            for tqb in range(NQ):
                # -- transpose this 512-row slice of x into xs[C-part, 512] --
                xs = xsp.tile([P, KO, TQB], bf16, tag="xs")
                for lt in range(4):
                    ti = 4 * tqb + lt
                    xn = xnp.tile([P, C], f32, tag="xn")
                    nc.sync.dma_start(xn[:], x_d.ap()[ts(ti, P), :])
                    xb = xnp.tile([P, C], bf16, tag="xb")
                    nc.vector.tensor_copy(xb[:], xn[:])
                    for kk in range(0, KO, 4):
                        pt = ps.tile([P, 512], bf16, tag="mm512")
                        for j in range(4):
                            nc.tensor.transpose(
                                pt[:, ts(j, P)], xb[:, ts(kk + j, P)], ident[:]
                            )
                        nc.vector.tensor_copy(
                            xs[:, kk : kk + 4, ts(lt, P)],
                            pt[:].rearrange("p (k t) -> p k t", k=4),
                        )

                # -- Q^T / K^T for this query block --
                for cc in range(2):
                    pq = ps.tile([P, TQB], f32, tag="mm512")
                    for ko in range(KO):
                        nc.tensor.matmul(
                            pq[:],
                            wq[:, ko, ts(cc, P)],
                            xs[:, ko, :],
                            start=(ko == 0),
                            stop=(ko == KO - 1),
                        )
                    nc.vector.tensor_scalar_add(
                        qt[:, cc, ts(tqb, TQB)], pq[:], bqt[:, cc : cc + 1]
                    )
                    pk = ps.tile([P, TQB], f32, tag="mm512")
                    for ko in range(KO):
                        nc.tensor.matmul(
                            pk[:],
                            wk[:, ko, ts(cc, P)],
                            xs[:, ko, :],
                            start=(ko == 0),
                            stop=(ko == KO - 1),
                        )
                    nc.vector.tensor_scalar_add(
                        kt[:, cc, ts(tqb, TQB)], pk[:], bkt[:, cc : cc + 1]
                    )

                # -- V for the 4 key blocks of this slice --
                for lt in range(4):
                    tk = 4 * tqb + lt
                    pv = ps.tile([P, HC], f32, tag="mm512")
                    for ko in range(KO):
                        nc.tensor.matmul(
                            pv[:],
                            xs[:, ko, ts(lt, P)],
                            wv[:, ko, :],
                            start=(ko == 0),
                            stop=(ko == KO - 1),
                        )
                    nc.vector.tensor_tensor(
                        v[:, tk, :, 0:D],
                        pv[:].rearrange("p (h a) -> p h a", h=NH),
                        bvt[:].rearrange("p (h a) -> p h a", h=NH),
                        ALU.add,
                    )

                # -- causal attention for both head pairs --
                # (interleaved with the previous round's deferred projection)
                for hp in range(2):
                    h0, h1 = 2 * hp, 2 * hp + 1
                    y0 = ps.tile([D + 1, TQB], f32, tag="y")
                    y1 = ps.tile([D + 1, TQB], f32, tag="y")
                    q0 = qt[0:64, hp, ts(tqb, TQB)]
                    q1 = qt[64:128, hp, ts(tqb, TQB)]
                    ntk = 4 * (tqb + 1)
                    for tkk in range(0, ntk, 2):
                        s0 = ps.tile([P, 2 * TQB], f32, tag="s")
                        s1 = ps.tile([P, 2 * TQB], f32, tag="s")
                        for j in range(2):
                            tk = tkk + j
                            nc.tensor.matmul(
                                s0[:, ts(j, TQB)],
                                kt[0:64, hp, ts(tk, P)],
                                q0,
                                start=True,
                                stop=True,
                                tile_position=(0, 0),
                            )
                            nc.tensor.matmul(
                                s1[:, ts(j, TQB)],
                                kt[64:128, hp, ts(tk, P)],
                                q1,
                                start=True,
                                stop=True,
                                tile_position=(64, 0),
                            )
                        e0 = ep.tile([P, 2 * TQB], bf16, tag="e")
                        e1 = ep.tile([P, 2 * TQB], bf16, tag="e")
                        nc.scalar.activation(e0[:], s0[:], AF.Exp, scale=SCALE)
                        nc.scalar.activation(e1[:], s1[:], AF.Exp, scale=SCALE)
                        for j in range(2):
                            tk = tkk + j
                            jd = tk - 4 * tqb  # diagonal strip index
                            if jd >= 0:
                                # cols < 128*jd are fully masked; the
                                # [128*jd, 128*(jd+1)) strip is triangular
                                for e in (e0, e1):
                                    if jd > 0:
                                        nc.vector.memset(
                                            e[:, j * TQB : j * TQB + jd * P], 0.0
                                        )
                                    st = j * TQB + jd * P
                                    nc.vector.tensor_mul(
                                        e[:, st : st + P], e[:, st : st + P], tri
                                    )
                        for j in range(2):
                            tk = tkk + j
                            nc.tensor.matmul(
                                y0[:],
                                v[:, tk, h0, :],
                                e0[:, ts(j, TQB)],
                                start=(tk == 0),
                                stop=(tk == ntk - 1),
                            )
                            nc.tensor.matmul(
                                y1[:],
                                v[:, tk, h1, :],
                                e1[:, ts(j, TQB)],
                                start=(tk == 0),
                                stop=(tk == ntk - 1),
                            )
                    den0 = rp.tile([1, TQB], f32, tag="den")
                    den1 = rp.tile([1, TQB], f32, tag="den")
                    nc.vector.tensor_copy(den0[:], y0[64:65, :])
                    nc.vector.tensor_copy(den1[:], y1[64:65, :])
                    rec0 = rp.tile([1, TQB], f32, tag="rec")
                    rec1 = rp.tile([1, TQB], f32, tag="rec")
                    nc.vector.reciprocal_approx_fast(rec0[:], den0[:])
                    nc.vector.reciprocal_approx_fast(rec1[:], den1[:])
                    rb0 = rp.tile([D, TQB], f32, tag="rb")
                    rb1 = rp.tile([D, TQB], f32, tag="rb")
                    nc.gpsimd.partition_broadcast(rb0[:], rec0[:])
                    nc.gpsimd.partition_broadcast(rb1[:], rec1[:])
                    nc.vector.tensor_mul(
                        yt[0:64, hp, ts(tqb, TQB)], y0[0:64, :], rb0[:]
                    )
                    nc.vector.tensor_mul(
                        yt[64:128, hp, ts(tqb, TQB)], y1[0:64, :], rb1[:]
                    )
                    for _ in range(4):
                        if pending_proj:
                            emit_proj(*pending_proj.pop(0))

                pending_proj += [
                    (mt, nb) for mt in range(4 * tqb, 4 * tqb + 4) for nb in range(2)
                ]

            for mt_nb in pending_proj:
                emit_proj(*mt_nb)

    nc.compile()
    return nc


def _get_nc():
    global _NC
    if _NC is None:
        _NC = _build()
    return _NC


def _shard(x, W_qkv, b_qkv, W_proj, b_proj):
    x = np.ascontiguousarray(np.asarray(x, dtype=np.float32))
    W_qkv = np.ascontiguousarray(np.asarray(W_qkv, dtype=np.float32))
    b_qkv = np.ascontiguousarray(np.asarray(b_qkv, dtype=np.float32))
    W_proj = np.ascontiguousarray(np.asarray(W_proj, dtype=np.float32))
    in_maps = []
    for core in range(8):
        b, g = core // 4, core % 4
        cs = slice(g * HC, (g + 1) * HC)
        in_maps.append(
            {
                "x": np.ascontiguousarray(x[b]),
                "wq": np.ascontiguousarray(
                    W_qkv[:, 0 * C :][:, cs].astype(ml_dtypes.bfloat16)
                ),
                "wk": np.ascontiguousarray(
                    W_qkv[:, 1 * C :][:, cs].astype(ml_dtypes.bfloat16)
                ),
                "wv": np.ascontiguousarray(
                    W_qkv[:, 2 * C :][:, cs].astype(ml_dtypes.bfloat16)
                ),
                "wp": np.ascontiguousarray(W_proj[cs, :].astype(ml_dtypes.bfloat16)),
                "bq": np.ascontiguousarray(b_qkv[0 * C :][cs]),
                "bk": np.ascontiguousarray(b_qkv[1 * C :][cs]),
                "bv": np.ascontiguousarray(b_qkv[2 * C :][cs]),
            }
        )
    return in_maps


def _gather(results, b_proj):
    b_proj = np.asarray(b_proj, dtype=np.float32)
    y = np.empty((2, T, C), dtype=np.float32)
    for b in range(2):
        acc = results[4 * b]["out"].astype(np.float32).copy()
        for g in range(1, 4):
            acc += results[4 * b + g]["out"]
        y[b] = acc + b_proj
    return y


def kernel(x, W_qkv, b_qkv, W_proj, b_proj):
    nc = _get_nc()
    in_maps = _shard(x, W_qkv, b_qkv, W_proj, b_proj)
    res = bass_utils.run_bass_kernel_spmd(nc, in_maps, core_ids=list(range(8)))
    return _gather(res.results, b_proj)
